# revision 2
# baseline (speedup 1.0000x reference)
"""Causal self-attention (B=4, T=2048, C=2048, H=16, hd=128) on 8 trn2 cores.

Sharding: core = b*2 + half. Each core handles batch b and 8 heads
(half*8 .. half*8+7). Each core computes a partial out-projection
(contribution of its 8 heads); host sums the two partials per batch.

v2 design (vs the fp32r baseline):
 - all-bf16 matmul datapath: same PE rate as wide fp32r, but no 4x
   narrow-width penalty, half the DMA bytes and SBUF footprint.
 - q/k/v/y fully SBUF-resident: no DRAM scratch roundtrip at all.
 - V projection first with contraction-outer loop (PE starts as soon as
   the first x chunk + wv land).
 - per-head software pipeline: head h's attention matmuls interleaved
   with head h+1's q/k projection strips so ACT exp latency/throughput
   hides under projection matmuls.
 - causal trimming of scores/exp/attV/rowsum to valid widths; diagonal
   masking via post-exp affine_select(fill=0) on the idle gpsimd engine.
 - softmax normalization: PE rowsum (ones-stationary) accumulated per
   strip, DVE reciprocal, gpsimd partition_broadcast, DVE multiply.
"""

import numpy as np

import concourse.bass as bass
import concourse.tile as tile
from concourse import bacc, bass2jax, mybir

F32 = mybir.dt.float32
F32R = mybir.dt.float32r
BF16 = mybir.dt.bfloat16
F16 = mybir.dt.float16

B = 4
T = 2048
C = 2048
HD = 128
HL = 8          # local heads per core
NCC = 16        # contraction chunks of 128 over C
NTB = 16        # t blocks of 128
NQS = 4         # q strips of 512
SW = 512
N_CORES = 8

SWAP = list(range(16, 32)) + list(range(16))
EXPF = mybir.ActivationFunctionType.Exp


def build_program(repeat=1):
    nc = bacc.Bacc(None, target_bir_lowering=False)

    xT = nc.declare_dram_parameter("xT", [NCC, 128, T], BF16, isOutput=False)
    wq = nc.declare_dram_parameter("wq", [HL, 128, C], BF16, isOutput=False)
    wk = nc.declare_dram_parameter("wk", [HL, 128, C], BF16, isOutput=False)
    wv = nc.declare_dram_parameter("wv", [2, 128, NCC * SW], BF16, isOutput=False)
    wp = nc.declare_dram_parameter("wp", [HL, 128, C], BF16, isOutput=False)
    cs = nc.declare_dram_parameter("cs", [128, T], F16, isOutput=False)
    ss = nc.declare_dram_parameter("ss", [128, T], F16, isOutput=False)
    out = nc.declare_dram_parameter("out", [T, C], F32, isOutput=True)

    with tile.TileContext(nc) as tc:
      for _rep in range(repeat):
        with tc.tile_pool(name="const", bufs=1) as cpool:
            ones_col = cpool.tile([128, 1], F16, name="ones_col", tag="oc")
            nc.vector.memset(ones_col[:], 1.0)
            cs_sb = cpool.tile([128, T], F16, name="cs_sb", tag="cs")
            ss_sb = cpool.tile([128, T], F16, name="ss_sb", tag="ss")
            ysb = [
                cpool.tile([128, T], BF16, name=f"ysb{h}", tag=f"y{h}")
                for h in range(HL)
            ]

            with (
                tc.tile_pool(name="vsp", bufs=1) as vspool,
                tc.tile_pool(name="qkh", bufs=2) as qkhpool,
            ):
                vs = [
                    vspool.tile(
                        [128, HL * 128], F16, name=f"vs{tb}", tag=f"v{tb}"
                    )
                    for tb in range(NTB)
                ]
                ctx = Emitter(nc, vs, ysb, ones_col, cs_sb, ss_sb)
                ctx.qkhpool = qkhpool
                ctx.wq, ctx.wk = wq, wk

                with (
                    tc.tile_pool(name="xin", bufs=1) as xpool,
                    tc.tile_pool(name="wqk0", bufs=1) as wqk0pool,
                ):
                    ctx.wqkpool = wqk0pool
                    ctx.wqk_bufs = 1
                    # wv qd0 first so the V phase's first matmul isn't
                    # behind the whole x transfer on the DMA engines
                    with (
                        tc.tile_pool(name="wvp", bufs=1) as wvpool,
                        tc.tile_pool(name="vps", bufs=1, space="PSUM")
                        as vpspool,
                    ):
                        wvsb = [
                            wvpool.tile(
                                [128, NCC * SW], BF16,
                                name=f"wv{qd}", tag=f"wv{qd}",
                            )
                            for qd in range(2)
                        ]
                        xsb = [
                            xpool.tile(
                                [128, T], BF16, name=f"xsb{cc}", tag=f"x{cc}"
                            )
                            for cc in range(NCC)
                        ]
                        # interleave wv-slice + x-half DMAs in exact
                        # consumption order of the cc-outer V loop: the
                        # first tb-group (tb 0..7) reads only x cols 0:1024
                        for cc in range(NCC):
                            nc.sync.dma_start(
                                out=wvsb[0][:, cc * SW:(cc + 1) * SW],
                                in_=wv[0][:, cc * SW:(cc + 1) * SW],
                            )
                            if cc == 0:
                                nc.sync.dma_start(
                                    out=xsb[0][:, 0:128], in_=xT[0][:, 0:128]
                                )
                                nc.sync.dma_start(
                                    out=xsb[0][:, 128:1024],
                                    in_=xT[0][:, 128:1024],
                                )
                            else:
                                nc.sync.dma_start(
                                    out=xsb[cc][:, 0:1024],
                                    in_=xT[cc][:, 0:1024],
                                )
                        for cc in range(NCC):
                            nc.sync.dma_start(
                                out=xsb[cc][:, 1024:T], in_=xT[cc][:, 1024:T]
                            )
                        nc.sync.dma_start(out=wvsb[1][:], in_=wv[1])
                        ctx.xsb = xsb
                        nc.sync.dma_start(out=cs_sb[:], in_=cs[:])
                        nc.sync.dma_start(out=ss_sb[:], in_=ss[:])
                        w_cur = ctx.emit_qk_weights(0)
                        # ---- V projection (cc-outer PSUM lane groups) ----
                        groups = [(0, [8, 8]), (1, [8, 5, 3])]
                        for qd, sizes in groups:
                            tb0 = 0
                            lb = 0
                            for gi, gsz in enumerate(sizes):
                                last = qd == 1 and gi == len(sizes) - 1
                                pvt = [
                                    vpspool.tile(
                                        [128, SW], F32, name="pv",
                                        tag=f"pv{(lb + ln) % 8}", bufs=1,
                                    )
                                    for ln in range(gsz)
                                ]
                                lb += gsz
                                for cc in range(NCC):
                                    for ln in range(gsz):
                                        tb = tb0 + ln
                                        nc.tensor.matmul(
                                            pvt[ln][:],
                                            xsb[cc][:, tb * 128:(tb + 1) * 128],
                                            wvsb[qd][:, cc * SW:(cc + 1) * SW],
                                            start=(cc == 0),
                                            stop=(cc == NCC - 1),
                                        )
                                for ln in range(gsz):
                                    tb = tb0 + ln
                                    dst = vs[tb][:, qd * SW:(qd + 1) * SW]
                                    if last and ln % 2 == 1:
                                        nc.vector.tensor_copy(dst, pvt[ln][:])
                                    else:
                                        nc.scalar.copy(out=dst, in_=pvt[ln][:])
                                tb0 += gsz

                    # ---- per-head pipeline (heads 0..6 attention) ----
                    with (
                        tc.tile_pool(name="stp", bufs=3, space="PSUM") as stpool,
                        tc.tile_pool(name="opp", bufs=2, space="PSUM") as oppool,
                        tc.tile_pool(name="smp", bufs=1, space="PSUM") as sumpool,
                        tc.tile_pool(name="esb", bufs=8) as epool,
                        tc.tile_pool(name="nrm", bufs=2) as npool,
                        tc.tile_pool(name="acp", bufs=2) as apool,
                    ):
                        ctx.set_p2_pools(
                            stpool, oppool, sumpool, epool, npool, apool
                        )
                        with (
                            tc.tile_pool(name="wqkr", bufs=2) as wqkrpool,
                            tc.tile_pool(name="pqk", bufs=2, space="PSUM")
                            as pqkpool,
                            tc.tile_pool(name="rt", bufs=2) as rtpool,
                        ):
                            ctx.pqkpool = pqkpool
                            ctx.rtpool = rtpool

                            qk_cur = ctx.emit_qk_head(w_cur)
                            ctx.wqkpool = wqkrpool
                            ctx.wqk_bufs = 2
                            for h in range(HL - 1):
                                nxt_w = ctx.emit_qk_weights(h + 1)
                                qk_cur = ctx.emit_slot(h, qk_cur, nxt_w)

                # ---- x freed: stage wp, then run head 7's attention in
                # fresh pools with the first 16 out-projection units (tb
                # 0..3, which need only head 7's g=0 strip) interleaved
                # into its ACT-bound windows; finish the projection after.
                with (
                    tc.tile_pool(name="wpp", bufs=1) as wppool,
                    tc.tile_pool(name="osb", bufs=4) as ospool,
                ):
                    wpsb = []
                    for cb in range(HL):
                        wt = wppool.tile(
                            [128, C], BF16, name=f"wp{cb}", tag=f"wp{cb}"
                        )
                        nc.sync.dma_start(out=wt[:], in_=wp[cb])
                        wpsb.append(wt)

                    def p3_finish(dst, tb, csi, pf):
                        if csi % 2 == 0:
                            nc.scalar.copy(out=dst, in_=pf[:])
                        else:
                            nc.vector.tensor_copy(dst, pf[:])
                        nc.sync.dma_start(
                            out=out[
                                tb * 128:(tb + 1) * 128,
                                csi * SW:(csi + 1) * SW,
                            ],
                            in_=dst,
                        )

                    with (
                        tc.tile_pool(name="stp2", bufs=3, space="PSUM")
                        as stpool2,
                        tc.tile_pool(name="opp2", bufs=2, space="PSUM")
                        as oppool2,
                        tc.tile_pool(name="smp2", bufs=1, space="PSUM")
                        as sumpool2,
                        tc.tile_pool(name="esb2", bufs=8) as epool2,
                        tc.tile_pool(name="nrm2", bufs=2) as npool2,
                        tc.tile_pool(name="acp2", bufs=2) as apool2,
                        tc.tile_pool(name="p3a", bufs=2, space="PSUM")
                        as p3apool,
                    ):
                        ctx.set_p2_pools(
                            stpool2, oppool2, sumpool2, epool2, npool2, apool2
                        )
                        units = [
                            (tb, csi) for tb in range(4) for csi in range(4)
                        ]
                        state = {"i": 0}

                        def filler(n):
                            for _ in range(n):
                                if state["i"] >= len(units):
                                    return
                                tb, csi = units[state["i"]]
                                state["i"] += 1
                                pf = p3apool.tile(
                                    [128, SW], F32, name="pfa",
                                    tag=f"pfa{state['i'] % 2}", bufs=1,
                                )
                                for cb in range(HL):
                                    nc.tensor.matmul(
                                        pf[:],
                                        ysb[cb][:, tb * 128:(tb + 1) * 128],
                                        wpsb[cb][:, csi * SW:(csi + 1) * SW],
                                        start=(cb == 0),
                                        stop=(cb == HL - 1),
                                    )
                                dst = ospool.tile(
                                    [128, SW], F32, name="osf", tag="osf",
                                    bufs=4,
                                )
                                p3_finish(dst[:], tb, csi, pf)

                        ctx.emit_slot(HL - 1, qk_cur, None, filler=filler)
                        filler(len(units))

                    # remaining out projection (tb 4..15), lane-outer so
                    # stops stagger and copies/DMAs pipeline
                    with tc.tile_pool(name="p3p", bufs=1, space="PSUM") as p3pool:
                        li = 0
                        for tbp in range(2, NTB // 2):
                            ots = [
                                ospool.tile(
                                    [128, C], F32, name="osb", tag="osb", bufs=2
                                )
                                for _ in range(2)
                            ]
                            for j in range(2):
                                tb = 2 * tbp + j
                                for csi in range(4):
                                    pf = p3pool.tile(
                                        [128, SW], F32, name="pf",
                                        tag=f"pf{li % 8}", bufs=1,
                                    )
                                    li += 1
                                    for cb in range(HL):
                                        nc.tensor.matmul(
                                            pf[:],
                                            ysb[cb][:, tb * 128:(tb + 1) * 128],
                                            wpsb[cb][:, csi * SW:(csi + 1) * SW],
                                            start=(cb == 0),
                                            stop=(cb == HL - 1),
                                        )
                                    p3_finish(
                                        ots[j][:, csi * SW:(csi + 1) * SW],
                                        tb, csi, pf,
                                    )

    nc.compile()
    return nc


class Emitter:
    def __init__(self, nc, vs, ysb, ones_col, cs_sb, ss_sb):
        self.nc = nc
        self.vs = vs
        self.ysb = ysb
        self.ones_col = ones_col
        self.cs_sb = cs_sb
        self.ss_sb = ss_sb
        self.xsb = None
        self.qkhpool = None
        self.wqkpool = None
        self.pqkpool = None
        self.rtpool = None
        self.wq = None
        self.wk = None

    lookahead = 2

    def set_p2_pools(self, stpool, oppool, sumpool, epool, npool, apool):
        self.stpool = stpool
        self.oppool = oppool
        self.sumpool = sumpool
        self.epool = epool
        self.npool = npool
        self.apool = apool

    def emit_qk_weights(self, h):
        nc = self.nc
        nb = self.wqk_bufs
        wqt = self.wqkpool.tile([128, C], BF16, name=f"wq{h}", tag="wq", bufs=nb)
        nc.sync.dma_start(out=wqt[:], in_=self.wq[h])
        wkt = self.wqkpool.tile([128, C], BF16, name=f"wk{h}", tag="wk", bufs=nb)
        nc.sync.dma_start(out=wkt[:], in_=self.wk[h])
        return (wqt, wkt)

    def strip_parts(self, wt, dst, s):
        """Returns (mm(c), rope()) emitters: 4 chunks of 4 accumulating
        matmuls into one PSUM bank, then RoPE on DVE into the bf16 SBUF
        destination strip."""
        nc = self.nc
        pqk = self.pqkpool.tile([128, SW], F32, name="pqk", tag="pqk", bufs=2)

        def mm(c):
            for cc in range(4 * c, 4 * c + 4):
                nc.tensor.matmul(
                    pqk[:],
                    wt[:, cc * 128:(cc + 1) * 128],
                    self.xsb[cc][:, s * SW:(s + 1) * SW],
                    start=(cc == 0),
                    stop=(cc == NCC - 1),
                )

        def rope():
            csl = self.cs_sb[:, s * SW:(s + 1) * SW]
            snl = self.ss_sb[:, s * SW:(s + 1) * SW]
            t1 = self.rtpool.tile([128, SW], F32, name="t1", tag="t1", bufs=2)
            nc.vector.tensor_mul(t1[:], pqk[:], csl)
            qsw = self.rtpool.tile([128, SW], F32, name="qsw", tag="qsw", bufs=2)
            nc.vector.stream_shuffle(qsw[:], pqk[:], SWAP)
            t2 = self.rtpool.tile([128, SW], F32, name="t2", tag="t2", bufs=2)
            nc.vector.tensor_mul(t2[:], qsw[:], snl)
            nc.vector.tensor_add(dst[:, s * SW:(s + 1) * SW], t1[:], t2[:])

        return mm, rope

    def emit_qk_strip(self, wt, dst, s):
        mm, rope = self.strip_parts(wt, dst, s)
        for c in range(4):
            mm(c)
        rope()

    def emit_qk_head(self, weights):
        """Unpipelined q/k projection (head 0 only)."""
        qt = self.qkhpool.tile([128, T], BF16, name="qh", tag="qh", bufs=2)
        kt = self.qkhpool.tile([128, T], BF16, name="kh", tag="kh", bufs=2)
        wqt, wkt = weights
        for s in range(NQS):
            self.emit_qk_strip(wqt, qt, s)
            self.emit_qk_strip(wkt, kt, s)
        return (qt, kt)

    def emit_scores(self, qk_cur, g, kb, blk):
        nc = self.nc
        qt, kt = qk_cur
        off = 128 * max(0, kb - 4 * g)
        pst = self.stpool.tile(
            [128, SW], F32, name="pst", tag="pst", bufs=self.lookahead + 1
        )
        nc.tensor.matmul(
            pst[:, off:SW],
            kt[:, kb * 128:(kb + 1) * 128],
            qt[:, g * SW + off:(g + 1) * SW],
            start=True,
            stop=True,
        )
        esb = self.epool.tile([128, SW], F16, name="esb", tag="esb", bufs=8)
        nc.scalar.activation(esb[:, off:SW], pst[:, off:SW], EXPF)
        if kb >= 4 * g:
            # diagonal block: zero the strictly-upper triangle (k > q)
            nc.gpsimd.affine_select(
                out=esb[:, off:off + 128],
                in_=esb[:, off:off + 128],
                compare_op=mybir.AluOpType.is_ge,
                fill=0.0,
                base=0,
                pattern=[[1, 128]],
                channel_multiplier=-1,
            )
        blk[kb] = (esb, off)

    def emit_normalize(self, h, g, po, sm):
        nc = self.nc
        recip = self.npool.tile([1, SW], F32, name="recip", tag="recip", bufs=2)
        nc.vector.reciprocal(recip[:], sm[0:1, :])
        bsb = self.npool.tile([128, SW], F32, name="bsb", tag="bsb", bufs=2)
        nc.gpsimd.partition_broadcast(bsb[:], recip[:])
        nc.vector.tensor_mul(
            self.ysb[h][:, g * SW:(g + 1) * SW], po[:], bsb[:]
        )

    def emit_slot(self, h, qk_cur, nxt_w, filler=None):
        """Attention for head h, interleaved with q/k projection strips of
        head h+1 (when nxt_w is given). Returns next head's (q, k) tiles."""
        nc = self.nc
        if nxt_w is not None:
            nqt = self.qkhpool.tile([128, T], BF16, name="qh", tag="qh", bufs=2)
            nkt = self.qkhpool.tile([128, T], BF16, name="kh", tag="kh", bufs=2)
            wqt, wkt = nxt_w
        pending = None

        def flush():
            nonlocal pending
            if pending is None:
                return
            pg, ppo, psm, pacc = pending
            pending = None
            nc.tensor.matmul(
                psm[0:1, :], self.ones_col[:], pacc[:], start=True, stop=True
            )
            self.emit_normalize(h, pg, ppo, psm)
            if filler is not None:
                filler(4)

        for g in range(NQS):
            nkb = 4 * g + 4
            diag = list(range(4 * g, nkb))
            full = list(range(0, 4 * g))
            blk = {}
            po = self.oppool.tile([128, SW], F32, name="po", tag="po", bufs=2)
            sm = self.sumpool.tile([1, SW], F32, name="sm", tag="sm", bufs=1)
            acc = self.apool.tile([128, SW], F16, name="acc", tag="acc", bufs=2)
            la = self.lookahead
            if nxt_w is not None:
                # diagonal blocks' scores go first: their exp + triangle
                # select (ACT -> Pool) complete under the projection strip,
                # so the diag attV chain is never exposed on the PE
                for kb in diag[:3]:
                    self.emit_scores(qk_cur, g, kb, blk)
                self.emit_qk_strip(wqt, nqt, g)
                self.emit_scores(qk_cur, g, diag[3], blk)
                for kb in full[:la]:
                    self.emit_scores(qk_cur, g, kb, blk)
                order = full + diag
            else:
                for kb in range(min(la, nkb)):
                    self.emit_scores(qk_cur, g, kb, blk)
                order = list(range(nkb))
            kmm = krope = None
            kpos = {}
            if nxt_w is not None and g >= 2:
                # g>=2 blocks are ACT-throughput-bound: feed the k strip's
                # matmul chunks into the late kb iterations as PE filler
                kmm, krope = self.strip_parts(wkt, nkt, g)
                kpos = {nkb - 8: 0, nkb - 6: 1, nkb - 4: 2, nkb - 2: 3}
            for i, kb in enumerate(order):
                esb, off = blk[kb]
                nc.tensor.matmul(
                    po[:, off:SW],
                    self.vs[kb][:, h * 128:(h + 1) * 128],
                    esb[:, off:SW],
                    start=(i == 0),
                    stop=(i == nkb - 1),
                )
                if i == 0:
                    nc.vector.tensor_copy(acc[:], esb[:])
                else:
                    nc.vector.tensor_add(
                        acc[:, off:SW], acc[:, off:SW], esb[:, off:SW]
                    )
                if i in kpos:
                    kmm(kpos[i])
                if nxt_w is not None:
                    if i + la < len(full):
                        self.emit_scores(qk_cur, g, full[i + la], blk)
                else:
                    if i + la < nkb:
                        self.emit_scores(qk_cur, g, order[i + la], blk)
            if nxt_w is not None:
                if krope is not None:
                    krope()
                else:
                    self.emit_qk_strip(wkt, nkt, g)
            pending = (g, po, sm, acc)
            flush()
        return (nqt, nkt) if nxt_w is not None else None


# ---------------------------------------------------------------------------
# Host-side preparation (identical math to the fp32r baseline, cast to bf16)

_PERM = np.concatenate(
    [
        np.concatenate([2 * (qd * 16 + np.arange(16)) + r for r in (0, 1)])
        for qd in range(4)
    ]
)
_PAIR_OF_SLOT = np.concatenate(
    [np.tile(qd * 16 + np.arange(16), 2) for qd in range(4)]
)
_SIN_SIGN = np.concatenate([np.repeat([-1.0, 1.0], 16) for _ in range(4)])

BF16_NP = mybir.dt.np(BF16)


def prepare_core_inputs(x, Wq, Wk, Wv, Wp):
    """Returns list of 8 input dicts, core = b*2 + half."""
    scale = 1.0 / np.sqrt(HD)

    inv_freq = (1.0 / (10000.0 ** (np.arange(0, HD, 2) / HD))).astype(np.float64)
    freqs = np.outer(inv_freq[_PAIR_OF_SLOT], np.arange(T, dtype=np.float64))
    F16_NP = mybir.dt.np(F16)
    cs = np.cos(freqs).astype(F16_NP)
    ss = (np.sin(freqs) * _SIN_SIGN[:, None]).astype(F16_NP)

    halves = []
    for half in range(2):
        r0 = half * HL * HD
        wq_in = np.empty((HL, 128, C), np.float32)
        wk_in = np.empty((HL, 128, C), np.float32)
        for h in range(HL):
            for arr, W, sc in ((wq_in, Wq, scale), (wk_in, Wk, 1.0)):
                Wh = W[r0 + h * HD: r0 + (h + 1) * HD][_PERM] * sc
                arr[h] = np.ascontiguousarray(
                    Wh.reshape(128, NCC, 128).transpose(2, 1, 0).reshape(128, C)
                )
        Wv_half = Wv[r0: r0 + HL * HD]
        wv_in = np.empty((2, 128, NCC * SW), np.float32)
        for qd in range(2):
            Wv4 = Wv_half[qd * SW:(qd + 1) * SW]
            wv_in[qd] = (
                Wv4.reshape(SW, NCC, 128).transpose(2, 1, 0).reshape(128, NCC * SW)
            )
        wp_in = np.ascontiguousarray(
            Wp.T[r0: r0 + HL * HD].reshape(HL, 128, C)
        )
        halves.append(
            (
                wq_in.astype(BF16_NP),
                wk_in.astype(BF16_NP),
                wv_in.astype(BF16_NP),
                wp_in.astype(BF16_NP),
            )
        )

    in_maps = []
    for b in range(B):
        xTb = np.ascontiguousarray(x[b].T).reshape(NCC, 128, T).astype(BF16_NP)
        for half in range(2):
            wq_in, wk_in, wv_in, wp_in = halves[half]
            in_maps.append(
                {
                    "xT": xTb,
                    "wq": wq_in,
                    "wk": wk_in,
                    "wv": wv_in,
                    "wp": wp_in,
                    "cs": cs,
                    "ss": ss,
                }
            )
    return in_maps


_RUNNER_CACHE = None


class _Runner:
    def __init__(self, sharded, mesh, in_names, out_names, out_avals, zero_shapes):
        self.sharded = sharded
        self.mesh = mesh
        self.in_names = in_names
        self.out_names = out_names
        self.out_avals = out_avals
        self.zero_shapes = zero_shapes
        self.body = None
        self.n_params = None
        self.donate = None

    def make_chained(self, n):
        """jit that runs the kernel n times back-to-back per dispatch,
        serialized by threading outputs into the next call's donated
        output operands."""
        import jax
        from jax.experimental.shard_map import shard_map
        from jax.sharding import PartitionSpec

        body, n_params = self.body, self.n_params

        def chained(*args):
            ins = args[:n_params]
            outs = tuple(args[n_params:])
            for _ in range(n):
                outs = body(*ins, *outs)
            return outs

        n_outs = len(self.out_names)
        in_specs = (PartitionSpec("core"),) * (n_params + n_outs)
        out_specs = (PartitionSpec("core"),) * n_outs
        return jax.jit(
            shard_map(
                chained,
                mesh=self.mesh,
                in_specs=in_specs,
                out_specs=out_specs,
                check_rep=False,
            ),
            donate_argnums=self.donate,
            keep_unused=True,
        )

    def concat_inputs(self, in_maps):
        return [
            np.concatenate([np.asarray(m[name]) for m in in_maps], axis=0)
            for name in self.in_names
        ]

    def make_zeros(self):
        return [np.zeros((N_CORES * s[0], *s[1:]), d) for (s, d) in self.zero_shapes]

    def run(self, in_maps):
        out_arrs = self.sharded(*self.concat_inputs(in_maps), *self.make_zeros())
        return [
            {
                name: np.asarray(out_arrs[i]).reshape(
                    N_CORES, *self.out_avals[i].shape
                )[c]
                for i, name in enumerate(self.out_names)
            }
            for c in range(N_CORES)
        ]


def _make_runner(nc=None):
    """Compile the Bass program once and return a _Runner that reuses the
    jitted executable across calls."""
    import jax
    from jax.experimental.shard_map import shard_map
    from jax.sharding import Mesh, PartitionSpec

    if nc is None:
        nc = build_program()
    bass2jax.install_neuronx_cc_hook()

    partition_name = nc.partition_id_tensor.name if nc.partition_id_tensor else None
    in_names, out_names, out_avals, zero_shapes = [], [], [], []
    for alloc in nc.m.functions[0].allocations:
        if not isinstance(alloc, mybir.MemoryLocationSet):
            continue
        name = alloc.memorylocations[0].name
        if alloc.kind == "ExternalInput":
            if name != partition_name:
                in_names.append(name)
        elif alloc.kind == "ExternalOutput":
            shape = tuple(alloc.tensor_shape)
            dtype = mybir.dt.np(alloc.dtype)
            out_names.append(name)
            out_avals.append(jax.core.ShapedArray(shape, dtype))
            zero_shapes.append((shape, dtype))
    n_params = len(in_names)
    n_outs = len(out_avals)
    all_in_names = list(in_names) + list(out_names)
    if partition_name is not None:
        all_in_names.append(partition_name)
    donate = tuple(range(n_params, n_params + n_outs))

    def _body(*args):
        operands = list(args)
        if partition_name is not None:
            operands.append(bass2jax.partition_id_tensor())
        outs = bass2jax._bass_exec_p.bind(
            *operands,
            out_avals=tuple(out_avals),
            in_names=tuple(all_in_names),
            out_names=tuple(out_names),
            lowering_input_output_aliases=(),
            sim_require_finite=True,
            sim_require_nnan=True,
            nc=nc,
        )
        return tuple(outs)

    devices = jax.devices()[:N_CORES]
    mesh = Mesh(np.asarray(devices), ("core",))
    in_specs = (PartitionSpec("core"),) * (n_params + n_outs)
    out_specs = (PartitionSpec("core"),) * n_outs
    sharded = jax.jit(
        shard_map(
            _body, mesh=mesh, in_specs=in_specs, out_specs=out_specs, check_rep=False
        ),
        donate_argnums=donate,
        keep_unused=True,
    )
    r = _Runner(sharded, mesh, in_names, out_names, out_avals, zero_shapes)
    r.body = _body
    r.n_params = n_params
    r.donate = donate
    return r


def get_runner():
    global _RUNNER_CACHE
    if _RUNNER_CACHE is None:
        _RUNNER_CACHE = _make_runner()
    return _RUNNER_CACHE


def kernel(x, Wq, Wk, Wv, Wp):
    runner = get_runner()
    in_maps = prepare_core_inputs(
        np.asarray(x), np.asarray(Wq), np.asarray(Wk), np.asarray(Wv), np.asarray(Wp)
    )
    res = runner.run(in_maps)
    out = np.empty((B, T, C), np.float32)
    for b in range(B):
        np.add(res[2 * b]["out"], res[2 * b + 1]["out"], out=out[b])
    return out


# revision 3
# speedup vs baseline: 1.0015x; 1.0015x over previous
"""Causal self-attention (B=4, T=2048, C=2048, H=16, hd=128) on 8 trn2 cores.

Sharding: core = b*2 + half. Each core handles batch b and 8 heads
(half*8 .. half*8+7). Each core computes a partial out-projection
(contribution of its 8 heads); host sums the two partials per batch.

v2 design (vs the fp32r baseline):
 - all-bf16 matmul datapath: same PE rate as wide fp32r, but no 4x
   narrow-width penalty, half the DMA bytes and SBUF footprint.
 - q/k/v/y fully SBUF-resident: no DRAM scratch roundtrip at all.
 - V projection first with contraction-outer loop (PE starts as soon as
   the first x chunk + wv land).
 - per-head software pipeline: head h's attention matmuls interleaved
   with head h+1's q/k projection strips so ACT exp latency/throughput
   hides under projection matmuls.
 - causal trimming of scores/exp/attV/rowsum to valid widths; diagonal
   masking via post-exp affine_select(fill=0) on the idle gpsimd engine.
 - softmax normalization: PE rowsum (ones-stationary) accumulated per
   strip, DVE reciprocal, gpsimd partition_broadcast, DVE multiply.
"""

import numpy as np

import concourse.bass as bass
import concourse.tile as tile
from concourse import bacc, bass2jax, mybir

F32 = mybir.dt.float32
F32R = mybir.dt.float32r
BF16 = mybir.dt.bfloat16
F16 = mybir.dt.float16

B = 4
T = 2048
C = 2048
HD = 128
HL = 8          # local heads per core
NCC = 16        # contraction chunks of 128 over C
NTB = 16        # t blocks of 128
NQS = 4         # q strips of 512
SW = 512
N_CORES = 8

SWAP = list(range(16, 32)) + list(range(16))
EXPF = mybir.ActivationFunctionType.Exp


def build_program(repeat=1):
    nc = bacc.Bacc(None, target_bir_lowering=False)

    xT = nc.declare_dram_parameter("xT", [NCC, 128, T], BF16, isOutput=False)
    wq = nc.declare_dram_parameter("wq", [HL, 128, C], BF16, isOutput=False)
    wk = nc.declare_dram_parameter("wk", [HL, 128, C], BF16, isOutput=False)
    wv = nc.declare_dram_parameter("wv", [2, 128, NCC * SW], BF16, isOutput=False)
    wp = nc.declare_dram_parameter("wp", [HL, 128, C], BF16, isOutput=False)
    cs = nc.declare_dram_parameter("cs", [128, T], F16, isOutput=False)
    ss = nc.declare_dram_parameter("ss", [128, T], F16, isOutput=False)
    out = nc.declare_dram_parameter("out", [T, C], F32, isOutput=True)

    with tile.TileContext(nc) as tc:
      for _rep in range(repeat):
        with tc.tile_pool(name="const", bufs=1) as cpool:
            ones_col = cpool.tile([128, 1], F16, name="ones_col", tag="oc")
            nc.vector.memset(ones_col[:], 1.0)
            cs_sb = cpool.tile([128, T], F16, name="cs_sb", tag="cs")
            ss_sb = cpool.tile([128, T], F16, name="ss_sb", tag="ss")
            ysb = [
                cpool.tile([128, T], BF16, name=f"ysb{h}", tag=f"y{h}")
                for h in range(HL)
            ]

            with (
                tc.tile_pool(name="vsp", bufs=1) as vspool,
                tc.tile_pool(name="qkh", bufs=2) as qkhpool,
            ):
                vs = [
                    vspool.tile(
                        [128, HL * 128], F16, name=f"vs{tb}", tag=f"v{tb}"
                    )
                    for tb in range(NTB)
                ]
                ctx = Emitter(nc, vs, ysb, ones_col, cs_sb, ss_sb)
                ctx.qkhpool = qkhpool
                ctx.wq, ctx.wk = wq, wk

                with (
                    tc.tile_pool(name="xin", bufs=1) as xpool,
                    tc.tile_pool(name="wqk0", bufs=1) as wqk0pool,
                ):
                    ctx.wqkpool = wqk0pool
                    ctx.wqk_bufs = 1
                    # wv qd0 first so the V phase's first matmul isn't
                    # behind the whole x transfer on the DMA engines
                    with (
                        tc.tile_pool(name="wvp", bufs=1) as wvpool,
                        tc.tile_pool(name="vps", bufs=1, space="PSUM")
                        as vpspool,
                    ):
                        wvsb = [
                            wvpool.tile(
                                [128, NCC * SW], BF16,
                                name=f"wv{qd}", tag=f"wv{qd}",
                            )
                            for qd in range(2)
                        ]
                        xsb = [
                            xpool.tile(
                                [128, T], BF16, name=f"xsb{cc}", tag=f"x{cc}"
                            )
                            for cc in range(NCC)
                        ]
                        # interleave wv-slice + x-half DMAs in exact
                        # consumption order of the cc-outer V loop: the
                        # first tb-group (tb 0..7) reads only x cols 0:1024
                        for cc in range(NCC):
                            nc.sync.dma_start(
                                out=wvsb[0][:, cc * SW:(cc + 1) * SW],
                                in_=wv[0][:, cc * SW:(cc + 1) * SW],
                            )
                            if cc == 0:
                                nc.sync.dma_start(
                                    out=xsb[0][:, 0:128], in_=xT[0][:, 0:128]
                                )
                                nc.sync.dma_start(
                                    out=xsb[0][:, 128:1024],
                                    in_=xT[0][:, 128:1024],
                                )
                            else:
                                nc.sync.dma_start(
                                    out=xsb[cc][:, 0:1024],
                                    in_=xT[cc][:, 0:1024],
                                )
                        for cc in range(NCC):
                            nc.sync.dma_start(
                                out=xsb[cc][:, 1024:T], in_=xT[cc][:, 1024:T]
                            )
                        nc.sync.dma_start(out=wvsb[1][:], in_=wv[1])
                        ctx.xsb = xsb
                        nc.sync.dma_start(out=cs_sb[:], in_=cs[:])
                        nc.sync.dma_start(out=ss_sb[:], in_=ss[:])
                        w_cur = ctx.emit_qk_weights(0)
                        # ---- V projection (cc-outer PSUM lane groups) ----
                        groups = [(0, [8, 8]), (1, [8, 5, 3])]
                        for qd, sizes in groups:
                            tb0 = 0
                            lb = 0
                            for gi, gsz in enumerate(sizes):
                                last = qd == 1 and gi == len(sizes) - 1
                                pvt = [
                                    vpspool.tile(
                                        [128, SW], F32, name="pv",
                                        tag=f"pv{(lb + ln) % 8}", bufs=1,
                                    )
                                    for ln in range(gsz)
                                ]
                                lb += gsz
                                for cc in range(NCC):
                                    for ln in range(gsz):
                                        tb = tb0 + ln
                                        nc.tensor.matmul(
                                            pvt[ln][:],
                                            xsb[cc][:, tb * 128:(tb + 1) * 128],
                                            wvsb[qd][:, cc * SW:(cc + 1) * SW],
                                            start=(cc == 0),
                                            stop=(cc == NCC - 1),
                                        )
                                for ln in range(gsz):
                                    tb = tb0 + ln
                                    dst = vs[tb][:, qd * SW:(qd + 1) * SW]
                                    if last and ln % 2 == 1:
                                        nc.vector.tensor_copy(dst, pvt[ln][:])
                                    else:
                                        nc.scalar.copy(out=dst, in_=pvt[ln][:])
                                tb0 += gsz

                    # ---- per-head pipeline (heads 0..6 attention) ----
                    with (
                        tc.tile_pool(name="stp", bufs=3, space="PSUM") as stpool,
                        tc.tile_pool(name="opp", bufs=2, space="PSUM") as oppool,
                        tc.tile_pool(name="smp", bufs=1, space="PSUM") as sumpool,
                        tc.tile_pool(name="esb", bufs=8) as epool,
                        tc.tile_pool(name="nrm", bufs=2) as npool,
                        tc.tile_pool(name="acp", bufs=2) as apool,
                    ):
                        ctx.set_p2_pools(
                            stpool, oppool, sumpool, epool, npool, apool
                        )
                        with (
                            tc.tile_pool(name="wqkr", bufs=2) as wqkrpool,
                            tc.tile_pool(name="pqk", bufs=2, space="PSUM")
                            as pqkpool,
                            tc.tile_pool(name="rt", bufs=2) as rtpool,
                        ):
                            ctx.pqkpool = pqkpool
                            ctx.rtpool = rtpool

                            qk_cur = ctx.emit_qk_head(w_cur)
                            ctx.wqkpool = wqkrpool
                            ctx.wqk_bufs = 2
                            for h in range(HL - 1):
                                nxt_w = ctx.emit_qk_weights(h + 1)
                                qk_cur = ctx.emit_slot(h, qk_cur, nxt_w)

                # ---- x freed: stage wp, then run head 7's attention in
                # fresh pools with the first 16 out-projection units (tb
                # 0..3, which need only head 7's g=0 strip) interleaved
                # into its ACT-bound windows; finish the projection after.
                with (
                    tc.tile_pool(name="wpp", bufs=1) as wppool,
                    tc.tile_pool(name="osb", bufs=4) as ospool,
                ):
                    wpsb = []
                    for cb in range(HL):
                        wt = wppool.tile(
                            [128, C], BF16, name=f"wp{cb}", tag=f"wp{cb}"
                        )
                        nc.sync.dma_start(out=wt[:], in_=wp[cb])
                        wpsb.append(wt)

                    def p3_finish(dst, tb, csi, pf, split=False):
                        if split:
                            # halve the tail latency of the very last chunk:
                            # the first half's DMA issues while the second
                            # half is still being copied
                            for hv in range(2):
                                lo, hi = hv * 256, (hv + 1) * 256
                                cp = nc.scalar.copy if hv == 0 else (
                                    lambda out, in_: nc.vector.tensor_copy(
                                        out, in_
                                    )
                                )
                                cp(out=dst[:, lo:hi], in_=pf[:, lo:hi])
                                nc.sync.dma_start(
                                    out=out[
                                        tb * 128:(tb + 1) * 128,
                                        csi * SW + lo:csi * SW + hi,
                                    ],
                                    in_=dst[:, lo:hi],
                                )
                            return
                        if csi % 2 == 0:
                            nc.scalar.copy(out=dst, in_=pf[:])
                        else:
                            nc.vector.tensor_copy(dst, pf[:])
                        nc.sync.dma_start(
                            out=out[
                                tb * 128:(tb + 1) * 128,
                                csi * SW:(csi + 1) * SW,
                            ],
                            in_=dst,
                        )

                    with (
                        tc.tile_pool(name="stp2", bufs=3, space="PSUM")
                        as stpool2,
                        tc.tile_pool(name="opp2", bufs=2, space="PSUM")
                        as oppool2,
                        tc.tile_pool(name="smp2", bufs=1, space="PSUM")
                        as sumpool2,
                        tc.tile_pool(name="esb2", bufs=8) as epool2,
                        tc.tile_pool(name="nrm2", bufs=2) as npool2,
                        tc.tile_pool(name="acp2", bufs=2) as apool2,
                        tc.tile_pool(name="p3a", bufs=2, space="PSUM")
                        as p3apool,
                    ):
                        ctx.set_p2_pools(
                            stpool2, oppool2, sumpool2, epool2, npool2, apool2
                        )
                        units = [
                            (tb, csi) for tb in range(4) for csi in range(4)
                        ]
                        state = {"i": 0}

                        def filler(n):
                            for _ in range(n):
                                if state["i"] >= len(units):
                                    return
                                tb, csi = units[state["i"]]
                                state["i"] += 1
                                pf = p3apool.tile(
                                    [128, SW], F32, name="pfa",
                                    tag=f"pfa{state['i'] % 2}", bufs=1,
                                )
                                for cb in range(HL):
                                    nc.tensor.matmul(
                                        pf[:],
                                        ysb[cb][:, tb * 128:(tb + 1) * 128],
                                        wpsb[cb][:, csi * SW:(csi + 1) * SW],
                                        start=(cb == 0),
                                        stop=(cb == HL - 1),
                                    )
                                dst = ospool.tile(
                                    [128, SW], F32, name="osf", tag="osf",
                                    bufs=4,
                                )
                                p3_finish(dst[:], tb, csi, pf)

                        ctx.emit_slot(HL - 1, qk_cur, None, filler=filler)
                        filler(len(units))

                    # remaining out projection (tb 4..15), lane-outer so
                    # stops stagger and copies/DMAs pipeline
                    with tc.tile_pool(name="p3p", bufs=1, space="PSUM") as p3pool:
                        li = 0
                        for tbp in range(2, NTB // 2):
                            ots = [
                                ospool.tile(
                                    [128, C], F32, name="osb", tag="osb", bufs=2
                                )
                                for _ in range(2)
                            ]
                            for j in range(2):
                                tb = 2 * tbp + j
                                for csi in range(4):
                                    pf = p3pool.tile(
                                        [128, SW], F32, name="pf",
                                        tag=f"pf{li % 8}", bufs=1,
                                    )
                                    li += 1
                                    for cb in range(HL):
                                        nc.tensor.matmul(
                                            pf[:],
                                            ysb[cb][:, tb * 128:(tb + 1) * 128],
                                            wpsb[cb][:, csi * SW:(csi + 1) * SW],
                                            start=(cb == 0),
                                            stop=(cb == HL - 1),
                                        )
                                    p3_finish(
                                        ots[j][:, csi * SW:(csi + 1) * SW],
                                        tb, csi, pf,
                                        split=(tbp == NTB // 2 - 1
                                               and j == 1 and csi == 3),
                                    )

    nc.compile()
    return nc


class Emitter:
    def __init__(self, nc, vs, ysb, ones_col, cs_sb, ss_sb):
        self.nc = nc
        self.vs = vs
        self.ysb = ysb
        self.ones_col = ones_col
        self.cs_sb = cs_sb
        self.ss_sb = ss_sb
        self.xsb = None
        self.qkhpool = None
        self.wqkpool = None
        self.pqkpool = None
        self.rtpool = None
        self.wq = None
        self.wk = None

    lookahead = 2

    def set_p2_pools(self, stpool, oppool, sumpool, epool, npool, apool):
        self.stpool = stpool
        self.oppool = oppool
        self.sumpool = sumpool
        self.epool = epool
        self.npool = npool
        self.apool = apool

    def emit_qk_weights(self, h):
        nc = self.nc
        nb = self.wqk_bufs
        wqt = self.wqkpool.tile([128, C], BF16, name=f"wq{h}", tag="wq", bufs=nb)
        nc.sync.dma_start(out=wqt[:], in_=self.wq[h])
        wkt = self.wqkpool.tile([128, C], BF16, name=f"wk{h}", tag="wk", bufs=nb)
        nc.sync.dma_start(out=wkt[:], in_=self.wk[h])
        return (wqt, wkt)

    def strip_parts(self, wt, dst, s):
        """Returns (mm(c), rope()) emitters: 4 chunks of 4 accumulating
        matmuls into one PSUM bank, then RoPE on DVE into the bf16 SBUF
        destination strip."""
        nc = self.nc
        pqk = self.pqkpool.tile([128, SW], F32, name="pqk", tag="pqk", bufs=2)

        def mm(c):
            for cc in range(4 * c, 4 * c + 4):
                nc.tensor.matmul(
                    pqk[:],
                    wt[:, cc * 128:(cc + 1) * 128],
                    self.xsb[cc][:, s * SW:(s + 1) * SW],
                    start=(cc == 0),
                    stop=(cc == NCC - 1),
                )

        def rope():
            csl = self.cs_sb[:, s * SW:(s + 1) * SW]
            snl = self.ss_sb[:, s * SW:(s + 1) * SW]
            t1 = self.rtpool.tile([128, SW], F32, name="t1", tag="t1", bufs=2)
            nc.vector.tensor_mul(t1[:], pqk[:], csl)
            qsw = self.rtpool.tile([128, SW], F32, name="qsw", tag="qsw", bufs=2)
            nc.vector.stream_shuffle(qsw[:], pqk[:], SWAP)
            t2 = self.rtpool.tile([128, SW], F32, name="t2", tag="t2", bufs=2)
            nc.vector.tensor_mul(t2[:], qsw[:], snl)
            nc.vector.tensor_add(dst[:, s * SW:(s + 1) * SW], t1[:], t2[:])

        return mm, rope

    def emit_qk_strip(self, wt, dst, s):
        mm, rope = self.strip_parts(wt, dst, s)
        for c in range(4):
            mm(c)
        rope()

    def emit_qk_head(self, weights):
        """Unpipelined q/k projection (head 0 only)."""
        qt = self.qkhpool.tile([128, T], BF16, name="qh", tag="qh", bufs=2)
        kt = self.qkhpool.tile([128, T], BF16, name="kh", tag="kh", bufs=2)
        wqt, wkt = weights
        for s in range(NQS):
            self.emit_qk_strip(wqt, qt, s)
            self.emit_qk_strip(wkt, kt, s)
        return (qt, kt)

    def emit_scores(self, qk_cur, g, kb, blk):
        nc = self.nc
        qt, kt = qk_cur
        off = 128 * max(0, kb - 4 * g)
        pst = self.stpool.tile(
            [128, SW], F32, name="pst", tag="pst", bufs=self.lookahead + 1
        )
        nc.tensor.matmul(
            pst[:, off:SW],
            kt[:, kb * 128:(kb + 1) * 128],
            qt[:, g * SW + off:(g + 1) * SW],
            start=True,
            stop=True,
        )
        esb = self.epool.tile([128, SW], F16, name="esb", tag="esb", bufs=8)
        nc.scalar.activation(esb[:, off:SW], pst[:, off:SW], EXPF)
        if kb >= 4 * g:
            # diagonal block: zero the strictly-upper triangle (k > q)
            nc.gpsimd.affine_select(
                out=esb[:, off:off + 128],
                in_=esb[:, off:off + 128],
                compare_op=mybir.AluOpType.is_ge,
                fill=0.0,
                base=0,
                pattern=[[1, 128]],
                channel_multiplier=-1,
            )
        blk[kb] = (esb, off)

    def emit_normalize(self, h, g, po, sm):
        nc = self.nc
        recip = self.npool.tile([1, SW], F32, name="recip", tag="recip", bufs=2)
        nc.vector.reciprocal(recip[:], sm[0:1, :])
        bsb = self.npool.tile([128, SW], F32, name="bsb", tag="bsb", bufs=2)
        nc.gpsimd.partition_broadcast(bsb[:], recip[:])
        nc.vector.tensor_mul(
            self.ysb[h][:, g * SW:(g + 1) * SW], po[:], bsb[:]
        )

    def emit_slot(self, h, qk_cur, nxt_w, filler=None):
        """Attention for head h, interleaved with q/k projection strips of
        head h+1 (when nxt_w is given). Returns next head's (q, k) tiles."""
        nc = self.nc
        if nxt_w is not None:
            nqt = self.qkhpool.tile([128, T], BF16, name="qh", tag="qh", bufs=2)
            nkt = self.qkhpool.tile([128, T], BF16, name="kh", tag="kh", bufs=2)
            wqt, wkt = nxt_w
        pending = None

        def flush():
            nonlocal pending
            if pending is None:
                return
            pg, ppo, psm, pacc = pending
            pending = None
            nc.tensor.matmul(
                psm[0:1, :], self.ones_col[:], pacc[:], start=True, stop=True
            )
            self.emit_normalize(h, pg, ppo, psm)
            if filler is not None:
                filler(4)

        for g in range(NQS):
            nkb = 4 * g + 4
            diag = list(range(4 * g, nkb))
            full = list(range(0, 4 * g))
            blk = {}
            po = self.oppool.tile([128, SW], F32, name="po", tag="po", bufs=2)
            sm = self.sumpool.tile([1, SW], F32, name="sm", tag="sm", bufs=1)
            acc = self.apool.tile([128, SW], F16, name="acc", tag="acc", bufs=2)
            la = self.lookahead
            if nxt_w is not None:
                # diagonal blocks' scores go first: their exp + triangle
                # select (ACT -> Pool) complete under the projection strip,
                # so the diag attV chain is never exposed on the PE
                for kb in diag[:3]:
                    self.emit_scores(qk_cur, g, kb, blk)
                self.emit_qk_strip(wqt, nqt, g)
                self.emit_scores(qk_cur, g, diag[3], blk)
                for kb in full[:la]:
                    self.emit_scores(qk_cur, g, kb, blk)
                order = full + diag
            else:
                for kb in range(min(la, nkb)):
                    self.emit_scores(qk_cur, g, kb, blk)
                order = list(range(nkb))
            kmm = krope = None
            kpos = {}
            if nxt_w is not None and g >= 2:
                # g>=2 blocks are ACT-throughput-bound: feed the k strip's
                # matmul chunks into the late kb iterations as PE filler
                kmm, krope = self.strip_parts(wkt, nkt, g)
                kpos = {nkb - 10: 0, nkb - 8: 1, nkb - 5: 2, nkb - 2: 3}
            acc_order = diag + full if nxt_w is not None else order
            for i, kb in enumerate(order):
                esb, off = blk[kb]
                nc.tensor.matmul(
                    po[:, off:SW],
                    self.vs[kb][:, h * 128:(h + 1) * 128],
                    esb[:, off:SW],
                    start=(i == 0),
                    stop=(i == nkb - 1),
                )
                akb = acc_order[i]
                aesb, aoff = blk[akb]
                if i == 0:
                    nc.vector.tensor_copy(acc[:], aesb[:])
                else:
                    nc.vector.tensor_add(
                        acc[:, aoff:SW], acc[:, aoff:SW], aesb[:, aoff:SW]
                    )
                if i in kpos:
                    kmm(kpos[i])
                if nxt_w is not None:
                    if i + la < len(full):
                        self.emit_scores(qk_cur, g, full[i + la], blk)
                else:
                    if i + la < nkb:
                        self.emit_scores(qk_cur, g, order[i + la], blk)
            if nxt_w is not None:
                if krope is not None:
                    krope()
                else:
                    self.emit_qk_strip(wkt, nkt, g)
            pending = (g, po, sm, acc)
            flush()
        return (nqt, nkt) if nxt_w is not None else None


# ---------------------------------------------------------------------------
# Host-side preparation (identical math to the fp32r baseline, cast to bf16)

_PERM = np.concatenate(
    [
        np.concatenate([2 * (qd * 16 + np.arange(16)) + r for r in (0, 1)])
        for qd in range(4)
    ]
)
_PAIR_OF_SLOT = np.concatenate(
    [np.tile(qd * 16 + np.arange(16), 2) for qd in range(4)]
)
_SIN_SIGN = np.concatenate([np.repeat([-1.0, 1.0], 16) for _ in range(4)])

BF16_NP = mybir.dt.np(BF16)


def prepare_core_inputs(x, Wq, Wk, Wv, Wp):
    """Returns list of 8 input dicts, core = b*2 + half."""
    scale = 1.0 / np.sqrt(HD)

    inv_freq = (1.0 / (10000.0 ** (np.arange(0, HD, 2) / HD))).astype(np.float64)
    freqs = np.outer(inv_freq[_PAIR_OF_SLOT], np.arange(T, dtype=np.float64))
    F16_NP = mybir.dt.np(F16)
    cs = np.cos(freqs).astype(F16_NP)
    ss = (np.sin(freqs) * _SIN_SIGN[:, None]).astype(F16_NP)

    halves = []
    for half in range(2):
        r0 = half * HL * HD
        wq_in = np.empty((HL, 128, C), np.float32)
        wk_in = np.empty((HL, 128, C), np.float32)
        for h in range(HL):
            for arr, W, sc in ((wq_in, Wq, scale), (wk_in, Wk, 1.0)):
                Wh = W[r0 + h * HD: r0 + (h + 1) * HD][_PERM] * sc
                arr[h] = np.ascontiguousarray(
                    Wh.reshape(128, NCC, 128).transpose(2, 1, 0).reshape(128, C)
                )
        Wv_half = Wv[r0: r0 + HL * HD]
        wv_in = np.empty((2, 128, NCC * SW), np.float32)
        for qd in range(2):
            Wv4 = Wv_half[qd * SW:(qd + 1) * SW]
            wv_in[qd] = (
                Wv4.reshape(SW, NCC, 128).transpose(2, 1, 0).reshape(128, NCC * SW)
            )
        wp_in = np.ascontiguousarray(
            Wp.T[r0: r0 + HL * HD].reshape(HL, 128, C)
        )
        halves.append(
            (
                wq_in.astype(BF16_NP),
                wk_in.astype(BF16_NP),
                wv_in.astype(BF16_NP),
                wp_in.astype(BF16_NP),
            )
        )

    in_maps = []
    for b in range(B):
        xTb = np.ascontiguousarray(x[b].T).reshape(NCC, 128, T).astype(BF16_NP)
        for half in range(2):
            wq_in, wk_in, wv_in, wp_in = halves[half]
            in_maps.append(
                {
                    "xT": xTb,
                    "wq": wq_in,
                    "wk": wk_in,
                    "wv": wv_in,
                    "wp": wp_in,
                    "cs": cs,
                    "ss": ss,
                }
            )
    return in_maps


_RUNNER_CACHE = None


class _Runner:
    def __init__(self, sharded, mesh, in_names, out_names, out_avals, zero_shapes):
        self.sharded = sharded
        self.mesh = mesh
        self.in_names = in_names
        self.out_names = out_names
        self.out_avals = out_avals
        self.zero_shapes = zero_shapes
        self.body = None
        self.n_params = None
        self.donate = None

    def make_chained(self, n):
        """jit that runs the kernel n times back-to-back per dispatch,
        serialized by threading outputs into the next call's donated
        output operands."""
        import jax
        from jax.experimental.shard_map import shard_map
        from jax.sharding import PartitionSpec

        body, n_params = self.body, self.n_params

        def chained(*args):
            ins = args[:n_params]
            outs = tuple(args[n_params:])
            for _ in range(n):
                outs = body(*ins, *outs)
            return outs

        n_outs = len(self.out_names)
        in_specs = (PartitionSpec("core"),) * (n_params + n_outs)
        out_specs = (PartitionSpec("core"),) * n_outs
        return jax.jit(
            shard_map(
                chained,
                mesh=self.mesh,
                in_specs=in_specs,
                out_specs=out_specs,
                check_rep=False,
            ),
            donate_argnums=self.donate,
            keep_unused=True,
        )

    def concat_inputs(self, in_maps):
        return [
            np.concatenate([np.asarray(m[name]) for m in in_maps], axis=0)
            for name in self.in_names
        ]

    def make_zeros(self):
        return [np.zeros((N_CORES * s[0], *s[1:]), d) for (s, d) in self.zero_shapes]

    def run(self, in_maps):
        out_arrs = self.sharded(*self.concat_inputs(in_maps), *self.make_zeros())
        return [
            {
                name: np.asarray(out_arrs[i]).reshape(
                    N_CORES, *self.out_avals[i].shape
                )[c]
                for i, name in enumerate(self.out_names)
            }
            for c in range(N_CORES)
        ]


def _make_runner(nc=None):
    """Compile the Bass program once and return a _Runner that reuses the
    jitted executable across calls."""
    import jax
    from jax.experimental.shard_map import shard_map
    from jax.sharding import Mesh, PartitionSpec

    if nc is None:
        nc = build_program()
    bass2jax.install_neuronx_cc_hook()

    partition_name = nc.partition_id_tensor.name if nc.partition_id_tensor else None
    in_names, out_names, out_avals, zero_shapes = [], [], [], []
    for alloc in nc.m.functions[0].allocations:
        if not isinstance(alloc, mybir.MemoryLocationSet):
            continue
        name = alloc.memorylocations[0].name
        if alloc.kind == "ExternalInput":
            if name != partition_name:
                in_names.append(name)
        elif alloc.kind == "ExternalOutput":
            shape = tuple(alloc.tensor_shape)
            dtype = mybir.dt.np(alloc.dtype)
            out_names.append(name)
            out_avals.append(jax.core.ShapedArray(shape, dtype))
            zero_shapes.append((shape, dtype))
    n_params = len(in_names)
    n_outs = len(out_avals)
    all_in_names = list(in_names) + list(out_names)
    if partition_name is not None:
        all_in_names.append(partition_name)
    donate = tuple(range(n_params, n_params + n_outs))

    def _body(*args):
        operands = list(args)
        if partition_name is not None:
            operands.append(bass2jax.partition_id_tensor())
        outs = bass2jax._bass_exec_p.bind(
            *operands,
            out_avals=tuple(out_avals),
            in_names=tuple(all_in_names),
            out_names=tuple(out_names),
            lowering_input_output_aliases=(),
            sim_require_finite=True,
            sim_require_nnan=True,
            nc=nc,
        )
        return tuple(outs)

    devices = jax.devices()[:N_CORES]
    mesh = Mesh(np.asarray(devices), ("core",))
    in_specs = (PartitionSpec("core"),) * (n_params + n_outs)
    out_specs = (PartitionSpec("core"),) * n_outs
    sharded = jax.jit(
        shard_map(
            _body, mesh=mesh, in_specs=in_specs, out_specs=out_specs, check_rep=False
        ),
        donate_argnums=donate,
        keep_unused=True,
    )
    r = _Runner(sharded, mesh, in_names, out_names, out_avals, zero_shapes)
    r.body = _body
    r.n_params = n_params
    r.donate = donate
    return r


def get_runner():
    global _RUNNER_CACHE
    if _RUNNER_CACHE is None:
        _RUNNER_CACHE = _make_runner()
    return _RUNNER_CACHE


def kernel(x, Wq, Wk, Wv, Wp):
    runner = get_runner()
    in_maps = prepare_core_inputs(
        np.asarray(x), np.asarray(Wq), np.asarray(Wk), np.asarray(Wv), np.asarray(Wp)
    )
    res = runner.run(in_maps)
    out = np.empty((B, T, C), np.float32)
    for b in range(B):
        np.add(res[2 * b]["out"], res[2 * b + 1]["out"], out=out[b])
    return out
